# revision 14
# baseline (speedup 1.0000x reference)
"""Trainium2 Bass kernel for nn_SGCNModel (signed GCN, 2 SignedConv layers + LN + proj head).

Strategy (8 NeuronCores, SPMD single NEFF):
  - Host relabels nodes (permutation) so each core owns NBLK*128 contiguous
    "padded" node ids; a repair pass bounds every (128-dst-block, sign,
    src-window) edge-bucket so the per-core control flow is IDENTICAL across
    cores; only data (indices / one-hot selectors) differs.
  - mean-aggregation = dma_gather (int16 row indices into <=32768-row source
    windows, 4 SWDGE queues) + segment-sum via one-hot matmuls into PSUM per
    128-dst block. The one-hot [128 edges x 128 dst] is built on DVE with a
    fused (iota == drel) * (1/deg) tensor_scalar op; mean division is folded.
  - Linearity: aggregation commutes with the linear layers, so features are
    pre-transformed (y = x @ W for layer 0; zP/zN = h0 @ Wz for layer 1),
    exchanged with AllGather, then gathered per edge.
  - On-chip layout is node-major [node, feat]; dense transforms use
    PE-transposed activation blocks as the stationary operand.
"""

import math
import os

import numpy as np

# ---------------------------------------------------------------------------
# problem config (FULL_CFG is the real problem; small cfgs used for sim tests)
# ---------------------------------------------------------------------------
FULL_CFG = dict(
    N=100000,    # real nodes
    E=800000,    # edges per sign
    NC=8,        # cores
    NBLK=98,     # 128-node dst blocks per core
    WSIZE=32768, # source window rows (int16 limit)
    CH0=4,       # blocks per layer-0 gather chunk
    CH1=2,       # blocks per layer-1 gather chunk
)

H = 128
H2 = 64
LN_EPS = 1e-5

LAST_EXEC_NS = None
LAST_RESULTS = None


def _windows(cfg):
    """Source windows over the padded node space: sizes and tiles-per-window."""
    npad = cfg["NC"] * cfg["NBLK"] * 128
    ws = cfg["WSIZE"]
    nfull = npad // ws
    sizes = [ws] * nfull
    rem = npad - nfull * ws
    if rem:
        sizes.append(rem)
    bases = [sum(sizes[:i]) for i in range(len(sizes))]
    # tiles per (block, window): enough for mean + 5 sigma; full windows get
    # equal share of the block's ~TPB*128 edges
    tpb_edges = 1280.0 * (cfg["E"] / 800000.0) * (98.0 / cfg["NBLK"]) * (8.0 / cfg["NC"])
    tpw = []
    for s in sizes:
        mean = tpb_edges * 0.8 * s / npad  # ~1024 * frac
        sig = math.sqrt(max(mean, 1.0))
        tpw.append(max(1, math.ceil((mean + 2.5 * sig + 8) / 128)))
    return bases, sizes, tpw


# ---------------------------------------------------------------------------
# host-side graph preprocessing
# ---------------------------------------------------------------------------
def _pack_nodes(pdeg, ndeg, pos, neg, cfg):
    """perm[orig] = new id. Snake round-robin by total degree, then repair so
    every (block, sign, window) bucket load <= 128*TPW[w]."""
    NC, NBLK = cfg["NC"], cfg["NBLK"]
    npad = NC * NBLK * 128
    nbins = NC * NBLK
    N = cfg["N"]
    deg = np.zeros(npad, np.int64)
    deg[:N] = pdeg + ndeg
    order = np.argsort(-deg, kind="stable")
    perm = np.empty(npad, np.int64)
    fwd = np.arange(nbins)
    rev = fwd[::-1]
    for s in range(128):
        nodes = order[s * nbins : (s + 1) * nbins]
        perm[nodes] = (fwd if s % 2 == 0 else rev) * 128 + s

    bases, sizes, tpw = _windows(cfg)
    nw = len(sizes)
    caps = np.array([128 * t for t in tpw], np.int64)
    wof = np.concatenate([[0], np.cumsum(sizes)])

    def win_of(newids):
        return np.searchsorted(wof, newids, side="right") - 1

    # per-node, per-(sign,window) in-degree under current source placement
    def node_loads():
        ld = np.zeros((npad, 2 * nw), np.int64)
        for si, ei in ((0, pos), (1, neg)):
            w = win_of(perm[ei[0]])
            np.add.at(ld, (ei[1], si * nw + w), 1)
        return ld

    ld = node_loads()
    inv = np.empty(npad, np.int64)
    inv[perm] = np.arange(npad)
    lim = np.concatenate([caps, caps])

    def bin_loads():
        bl = np.zeros((nbins, 2 * nw), np.int64)
        np.add.at(bl, perm // 128, ld)
        return bl

    binld = bin_loads()
    for _ in range(400):  # repair: swap one node per iteration
        bad = np.argwhere(binld > lim[None, :])
        if len(bad) == 0:
            break
        b, d = bad[np.argmax(binld[bad[:, 0], bad[:, 1]] - lim[bad[:, 1]])]
        members = inv[b * 128 : (b + 1) * 128]
        v = members[np.argmax(ld[members, d])]
        # target bin in the same pi-window (keeps all source windows intact)
        vwin = win_of(np.array([perm[v]]))[0]
        cand = np.arange(wof[vwin] // 128, wof[vwin + 1] // 128)
        cand = cand[cand != b]
        tb = cand[np.argmin(binld[cand, d])]
        tmembers = inv[tb * 128 : (tb + 1) * 128]
        u = tmembers[np.argmin(ld[tmembers, d])]
        pv, pu = perm[v], perm[u]
        perm[v], perm[u] = pu, pv
        inv[pu], inv[pv] = v, u
        binld[b] += ld[u] - ld[v]
        binld[tb] += ld[v] - ld[u]

    # final verification against the real edge lists
    binld = np.zeros((nbins, 2 * nw), np.int64)
    for si, ei in ((0, pos), (1, neg)):
        w = win_of(perm[ei[0]])
        np.add.at(binld, (perm[ei[1]] // 128, si * nw + w), 1)
    if (binld > lim[None, :]).any():
        raise RuntimeError("bucket cap violated after repair")
    return perm


def _chunks(cfg):
    NBLK = cfg["NBLK"]
    out = []
    for name, ch in (("l0", cfg["CH0"]), ("l1", cfg["CH1"])):
        ck = []
        b = 0
        while b < NBLK:
            ck.append((b, min(ch, NBLK - b)))
            b += ch
        out.append(ck)
    return out  # [l0_chunks, l1_chunks] as (start_block, nblocks)


def _slot_edges(src_new, dst_new, rc_edge, cfg):
    """Per-core slot assignment in the chunked/window-grouped tile layout.

    For each layer's chunking, produces:
      idx  int16 [128, ncols16]  (per-call [16, n/16] wrap, replicated x8)
      drel f32  [128, T]
      rc   f32  [128, T]
      where T = NBLK * TPB tile columns in chunk-major window-grouped order.
    """
    NC, NBLK = cfg["NC"], cfg["NBLK"]
    nsh = NBLK * 128
    bases, sizes, tpw = _windows(cfg)
    nw = len(sizes)
    TPB = sum(tpw)
    T = NBLK * TPB
    wof = np.concatenate([[0], np.cumsum(sizes)])
    chunks_all = _chunks(cfg)

    core_of = dst_new // nsh
    res = []
    for c in range(NC):
        m = core_of == c
        s = src_new[m]
        d = dst_new[m]
        rc = rc_edge[m]
        w = np.searchsorted(wof, s, side="right") - 1
        blk = (d % nsh) // 128
        o = np.lexsort((s, w, blk))
        s, d, rc, w, blk = s[o], d[o], rc[o], w[o], blk[o]
        # rank within (blk, w) bucket
        key = blk * nw + w
        cnt = np.bincount(key, minlength=NBLK * nw)
        base = np.zeros(NBLK * nw, np.int64)
        base[1:] = np.cumsum(cnt)[:-1]
        rank = np.arange(len(key)) - base[key]
        # tile-within-bucket and lane
        j = rank // 128
        p = rank % 128
        if (j >= np.array(tpw)[w]).any():
            raise RuntimeError("bucket overflow in slotting")

        # global tile column in BUCKET-CANONICAL order: blk*TPB + woff[w] + j
        woff = np.concatenate([[0], np.cumsum(tpw)])[:-1]
        col_canon = blk * TPB + woff[w] + j

        # canonical [T,128] slot tables
        idxv = np.zeros((T, 128), np.int64)
        drel = -np.ones((T, 128), np.float32)
        rcs = np.zeros((T, 128), np.float32)
        idxv[col_canon, p] = s - wof[w]
        drel[col_canon, p] = (d % 128).astype(np.float32)
        rcs[col_canon, p] = rc

        per_layer = []
        for chunks in chunks_all:
            # chunk-major window-grouped column order; also build the
            # per-call int16 wrapped index stream
            colmap = []  # new col -> canonical col
            call_sizes = []
            for (b0, nb) in chunks:
                for wi in range(nw):
                    ncols = nb * tpw[wi]
                    for bl in range(nb):
                        for jj in range(tpw[wi]):
                            colmap.append((b0 + bl) * TPB + woff[wi] + jj)
                    call_sizes.append(ncols * 128)
            colmap = np.array(colmap, np.int64)
            drel_l = np.ascontiguousarray(drel[colmap].T.astype(np.float32))
            rc_l = np.ascontiguousarray(rcs[colmap].T.astype(np.float32))
            # idx stream: per call, slots in (tile-local, lane) order
            iv = idxv[colmap]  # [T, 128] in new col order
            stream = iv.reshape(-1)  # slot i of call sequence, concatenated
            # wrap per call: [16, n/16]
            segs = []
            off = 0
            for n_call in call_sizes:
                seg = stream[off : off + n_call].reshape(n_call // 16, 16).T
                segs.append(seg)
                off += n_call
            wrapped = np.concatenate(segs, axis=1)  # [16, total/16]
            if wrapped.max() > 32767 or wrapped.min() < 0:
                raise RuntimeError("idx out of int16 range")
            idx16 = np.tile(wrapped.astype(np.int16), (8, 1))  # [128, total/16]
            per_layer.append(dict(idx=idx16, drel=drel_l, rc=rc_l))
        res.append(per_layer)
    return res


def _prep(inputs, cfg):
    NC, NBLK = cfg["NC"], cfg["NBLK"]
    N = cfg["N"]
    nsh = NBLK * 128
    npad = NC * nsh

    x = np.asarray(inputs["x"], np.float32)
    pos = np.asarray(inputs["pos_edge_index"], np.int64)
    neg = np.asarray(inputs["neg_edge_index"], np.int64)

    pdeg = np.bincount(pos[1], minlength=N)
    ndeg = np.bincount(neg[1], minlength=N)
    perm = _pack_nodes(pdeg, ndeg, pos, neg, cfg)

    rc_p = (1.0 / np.maximum(pdeg, 1)).astype(np.float32)
    rc_n = (1.0 / np.maximum(ndeg, 1)).astype(np.float32)

    slots_p = _slot_edges(perm[pos[0]], perm[pos[1]], rc_p[pos[1]], cfg)
    slots_n = _slot_edges(perm[neg[0]], perm[neg[1]], rc_n[neg[1]], cfg)

    x_pad = np.zeros((npad, H), np.float32)
    x_pad[perm[:N]] = x

    g = float(np.asarray(inputs["gate"], np.float32))

    def T_(w):
        return np.ascontiguousarray(np.asarray(w, np.float32).T)

    def bc(v):
        return np.ascontiguousarray(
            np.broadcast_to(np.asarray(v, np.float32).reshape(1, -1), (128, H))
        )

    w_y = np.concatenate([T_(inputs["c0_pl_w"]), T_(inputs["c0_nl_w"])], axis=1)
    w_l0pr = np.concatenate([T_(inputs["c0_pr_w"]), T_(inputs["c0_nr_w"])], axis=1)
    c1pl_T = T_(inputs["c1_pl_w"])
    c1nl_T = T_(inputs["c1_nl_w"])
    w_zp = np.zeros((H, H), np.float32)
    w_zp[0:H2, 0:H2] = c1pl_T[0:H2]
    w_zp[H2:H, H2:H] = c1nl_T[0:H2]
    w_zn = np.zeros((H, H), np.float32)
    w_zn[H2:H, 0:H2] = c1pl_T[H2:H]
    w_zn[0:H2, H2:H] = c1nl_T[H2:H]
    w_l1pr = np.zeros((H, H), np.float32)
    w_l1pr[0:H2, 0:H2] = T_(inputs["c1_pr_w"])
    w_l1pr[H2:H, H2:H] = T_(inputs["c1_nr_w"])

    consts = dict(
        w_y=w_y,
        w_l0pr=w_l0pr,
        w_zp=w_zp,
        w_zn=w_zn,
        w_l1pr=w_l1pr,
        w_p1=T_(inputs["pw1"]),
        w_p2=(T_(inputs["pw2"]) * g).astype(np.float32),
        bias0_bc=bc(np.concatenate([inputs["c0_pr_b"], inputs["c0_nr_b"]])),
        bias1_bc=bc(np.concatenate([inputs["c1_pr_b"], inputs["c1_nr_b"]])),
        g0_bc=bc(inputs["ln0_g"]),
        b0_bc=bc(inputs["ln0_b"]),
        g1_bc=bc(inputs["ln1_g"]),
        b1_bc=bc(inputs["ln1_b"]),
        pb1_bc=bc(inputs["pb1"]),
        pb2_bc=bc(np.asarray(inputs["pb2"], np.float32) * g),
        iota=np.ascontiguousarray(
            np.broadcast_to(np.arange(128, dtype=np.float32).reshape(1, -1), (128, 128))
        ),
        ident=np.eye(128, dtype=np.float32),
    )

    in_maps = []
    for c in range(NC):
        m = dict(consts)
        m["x"] = np.ascontiguousarray(x_pad[c * nsh : (c + 1) * nsh])
        for li, lname in ((0, "0"), (1, "1")):
            m[f"idx_p{lname}"] = slots_p[c][li]["idx"]
            m[f"idx_n{lname}"] = slots_n[c][li]["idx"]
            m[f"drel_p{lname}"] = slots_p[c][li]["drel"]
            m[f"drel_n{lname}"] = slots_n[c][li]["drel"]
            m[f"rc_p{lname}"] = slots_p[c][li]["rc"]
            m[f"rc_n{lname}"] = slots_n[c][li]["rc"]
        in_maps.append(m)

    return in_maps, perm, (1.0 - g)


# ---------------------------------------------------------------------------
# device kernel
# ---------------------------------------------------------------------------
def _build(cfg, one_minus_gate):
    import contextlib

    import concourse.bacc as bacc
    import concourse.tile as tile
    from concourse import bass, mybir
    from concourse._compat import axon_active

    NC, NBLK = cfg["NC"], cfg["NBLK"]
    nsh = NBLK * 128
    npad = NC * nsh
    bases, sizes, tpw = _windows(cfg)
    nw = len(sizes)
    TPB = sum(tpw)
    T = NBLK * TPB
    woff = [0]
    for t in tpw:
        woff.append(woff[-1] + t)
    chunks_all = _chunks(cfg)
    ncols16 = T * 128 // 16

    f32 = mybir.dt.float32
    i16 = mybir.dt.int16
    AF = mybir.ActivationFunctionType
    OP = mybir.AluOpType

    nc = bacc.Bacc(
        "TRN2",
        target_bir_lowering=False,
        debug=not axon_active(),
        num_devices=NC,
        num_swdge_queues=4,
    )

    x_in = nc.dram_tensor("x", [nsh, H], f32, kind="ExternalInput")
    dram_in = {}
    for li in ("0", "1"):
        for sgn in ("p", "n"):
            dram_in[f"idx_{sgn}{li}"] = nc.dram_tensor(
                f"idx_{sgn}{li}", [128, ncols16], i16, kind="ExternalInput"
            )
            dram_in[f"drel_{sgn}{li}"] = nc.dram_tensor(
                f"drel_{sgn}{li}", [128, T], f32, kind="ExternalInput"
            )
            dram_in[f"rc_{sgn}{li}"] = nc.dram_tensor(
                f"rc_{sgn}{li}", [128, T], f32, kind="ExternalInput"
            )
    const_names = [
        "w_y", "w_l0pr", "w_zp", "w_zn", "w_l1pr", "w_p1", "w_p2",
        "bias0_bc", "bias1_bc", "g0_bc", "b0_bc", "g1_bc", "b1_bc",
        "pb1_bc", "pb2_bc", "iota", "ident",
    ]
    for nm in const_names:
        dram_in[nm] = nc.dram_tensor(nm, [128, 128], f32, kind="ExternalInput")
    out_t = nc.dram_tensor("out", [nsh, H], f32, kind="ExternalOutput")

    rg = [list(range(NC))]
    qrr = [0]

    with tile.TileContext(nc) as tc:
        est = contextlib.ExitStack()
        with est:
            dram = est.enter_context(tc.tile_pool(name="dram", bufs=1, space="DRAM"))
            singles = est.enter_context(tc.tile_pool(name="singles", bufs=1))
            work = est.enter_context(tc.tile_pool(name="work", bufs=3))
            hwork = est.enter_context(tc.tile_pool(name="hwork", bufs=3))
            stats = est.enter_context(tc.tile_pool(name="stats", bufs=4))
            pre_pool = est.enter_context(tc.tile_pool(name="pre", bufs=NBLK))
            oh_pool = est.enter_context(tc.tile_pool(name="oh", bufs=6))
            ps_seg = est.enter_context(tc.tile_pool(name="ps_seg", bufs=4, space="PSUM"))
            ps_tp = est.enter_context(tc.tile_pool(name="ps_tp", bufs=2, space="PSUM"))
            ps_dn = est.enter_context(tc.tile_pool(name="ps_dn", bufs=2, space="PSUM"))

            y_p_own = dram.tile([nsh, H2], f32)
            y_n_own = dram.tile([nsh, H2], f32)
            y_p_full = dram.tile([npad, H2], f32, addr_space="Shared")
            y_n_full = dram.tile([npad, H2], f32, addr_space="Shared")
            zp_own = dram.tile([nsh, H], f32)
            zn_own = dram.tile([nsh, H], f32)
            zp_full = dram.tile([npad, H], f32, addr_space="Shared")
            zn_full = dram.tile([npad, H], f32, addr_space="Shared")

            sb = {}
            for nm in const_names:
                t = singles.tile([128, 128], f32, name=f"sb_{nm}")
                nc.sync.dma_start(out=t, in_=dram_in[nm][:, :])
                sb[nm] = t
            eps_t = singles.tile([128, 1], f32)
            nc.vector.memset(eps_t, LN_EPS)

            def load_ctrl(pool, li):
                for sgn in ("p", "n"):
                    for kind, dt, ncol in (
                        ("idx", i16, ncols16), ("drel", f32, T), ("rc", f32, T)
                    ):
                        nm = f"{kind}_{sgn}{li}"
                        t = pool.tile([128, ncol], dt, name=f"sb_{nm}")
                        nc.sync.dma_start(out=t, in_=dram_in[nm][:, :])
                        sb[nm] = t

            def next_q():
                q = qrr[0]
                qrr[0] = (q + 1) % 4
                return q

            # ---------------- phase A: y + pre_l0
            pre_tiles = []
            for b in range(NBLK):
                x_sb = work.tile([128, H], f32, name="x_sb", tag="x_sb")
                nc.sync.dma_start(out=x_sb, in_=x_in[b * 128 : (b + 1) * 128, :])
                ps_t = ps_tp.tile([128, 128], f32, name="ps_xt", tag="ps_t")
                nc.tensor.transpose(out=ps_t[:, :], in_=x_sb[:, :], identity=sb["ident"][:, :])
                xT = work.tile([128, 128], f32, name="xT", tag="xT")
                nc.scalar.copy(out=xT[:, :], in_=ps_t[:, :])
                ps_y = ps_dn.tile([128, H], f32, name="ps_y", tag="ps_dn")
                nc.tensor.matmul(out=ps_y[:, :], lhsT=xT[:, :], rhs=sb["w_y"][:, :], start=True, stop=True)
                y_sb = work.tile([128, H], f32, name="y_sb", tag="y_sb")
                nc.scalar.copy(out=y_sb[:, :], in_=ps_y[:, :])
                nc.sync.dma_start(out=y_p_own[b * 128 : (b + 1) * 128, :], in_=y_sb[:, 0:H2])
                nc.sync.dma_start(out=y_n_own[b * 128 : (b + 1) * 128, :], in_=y_sb[:, H2:H])
                ps_pr = ps_dn.tile([128, H], f32, name="ps_pr", tag="ps_dn")
                nc.tensor.matmul(out=ps_pr[:, :], lhsT=xT[:, :], rhs=sb["w_l0pr"][:, :], start=True, stop=True)
                pre_b = pre_pool.tile([128, H], f32, name="pre_b", tag="pre")
                nc.vector.tensor_tensor(out=pre_b[:, :], in0=ps_pr[:, :], in1=sb["bias0_bc"][:, :], op=OP.add)
                pre_tiles.append(pre_b)

            # ---------------- phase B: allgather y
            nc.gpsimd.collective_compute(
                "AllGather", OP.bypass, replica_groups=rg,
                ins=[y_p_own[:, :]], outs=[y_p_full[:, :]],
            )
            nc.gpsimd.collective_compute(
                "AllGather", OP.bypass, replica_groups=rg,
                ins=[y_n_own[:, :]], outs=[y_n_full[:, :]],
            )

            def gathers(msgs, srcs, idx_name, c16_ofs, nb, F):
                """One chunk's gather calls (one per window) into msgs."""
                ofs = c16_ofs
                tcol = 0
                for wi in range(nw):
                    ntile = nb * tpw[wi]
                    nidx = ntile * 128
                    nc.gpsimd.dma_gather(
                        out_ap=msgs[:, tcol : tcol + ntile, :],
                        in_ap=srcs[wi],
                        idxs_ap=sb[idx_name][:, ofs : ofs + nidx // 16],
                        num_idxs=nidx,
                        num_idxs_reg=nidx,
                        elem_size=F,
                        single_packet=False,
                        queue_num=next_q(),
                    )
                    ofs += nidx // 16
                    tcol += ntile
                return ofs

            def seg_mms(ps, msgs, bl, nb, sgn, li, cbase, col0, ncol, first, last):
                """TPB one-hot matmuls of one (block, sign) into psum cols.

                msgs columns are window-grouped for the chunk; tile j of
                window wi for local block bl sits at column
                woff_chunk(wi)*nb + bl*tpw[wi] + j. drel/rc columns are at
                cbase + (same local layout).
                """
                k = 0
                ccol = 0
                for wi in range(nw):
                    for j in range(tpw[wi]):
                        mcol = ccol + bl * tpw[wi] + j
                        dcol = cbase + ccol + bl * tpw[wi] + j
                        oh = oh_pool.tile([128, 128], f32, name="oh", tag="oh")
                        nc.vector.tensor_scalar(
                            out=oh[:, :],
                            in0=sb["iota"][:, :],
                            scalar1=sb[f"drel_{sgn}{li}"][:, dcol : dcol + 1],
                            scalar2=sb[f"rc_{sgn}{li}"][:, dcol : dcol + 1],
                            op0=OP.is_equal,
                            op1=OP.mult,
                        )
                        nc.tensor.matmul(
                            out=ps[:, col0 : col0 + ncol],
                            lhsT=oh[:, :],
                            rhs=msgs[:, mcol, 0:ncol],
                            start=(first and k == 0),
                            stop=(last and k == TPB - 1),
                        )
                        k += 1
                    ccol += nb * tpw[wi]

            def layer_norm(a, g_bc, b_bc, out_tile):
                mv6 = stats.tile([128, 6], f32, name="mv6", tag="mv6")
                nc.vector.bn_stats(out=mv6[:, :], in_=a[:, :])
                mv = stats.tile([128, 2], f32, name="mv", tag="mv")
                nc.vector.bn_aggr(out=mv[:, :], in_=mv6[:, :])
                sd = stats.tile([128, 1], f32, name="sd", tag="sd")
                nc.scalar.activation(
                    out=sd[:, :], in_=mv[:, 1:2], func=AF.Sqrt, bias=eps_t[:, :], scale=1.0
                )
                rs = stats.tile([128, 1], f32, name="rs", tag="rs")
                nc.vector.reciprocal(out=rs[:, :], in_=sd[:, :])
                tn = hwork.tile([128, H], f32, name="tn", tag="tn")
                nc.vector.tensor_scalar(
                    out=tn[:, :], in0=a[:, :],
                    scalar1=mv[:, 0:1], scalar2=rs[:, :],
                    op0=OP.subtract, op1=OP.mult,
                )
                tg = hwork.tile([128, H], f32, name="tg", tag="tg")
                nc.vector.tensor_tensor(out=tg[:, :], in0=tn[:, :], in1=g_bc[:, :], op=OP.mult)
                nc.vector.tensor_tensor(out=out_tile[:, :], in0=tg[:, :], in1=b_bc[:, :], op=OP.add)

            # ---------------- phase C+D: layer-0
            with tc.tile_pool(name="ctrl0", bufs=1) as ctrl0, tc.tile_pool(
                name="msgs0", bufs=1
            ) as msgs0:
                load_ctrl(ctrl0, "0")
                ysrc_p = [y_p_full[bases[wi] : bases[wi] + sizes[wi], :] for wi in range(nw)]
                ysrc_n = [y_n_full[bases[wi] : bases[wi] + sizes[wi], :] for wi in range(nw)]
                ofs_p = 0
                ofs_n = 0
                cbase = 0
                for (b0, nb) in chunks_all[0]:
                    mp_ = msgs0.tile([128, nb * TPB, H2], f32, name="m0p", tag="m0p", bufs=2)
                    mn_ = msgs0.tile([128, nb * TPB, H2], f32, name="m0n", tag="m0n", bufs=2)
                    new_ofs_p = gathers(mp_, ysrc_p, "idx_p0", ofs_p, nb, H2)
                    new_ofs_n = gathers(mn_, ysrc_n, "idx_n0", ofs_n, nb, H2)
                    ofs_p, ofs_n = new_ofs_p, new_ofs_n
                    for bl in range(nb):
                        b = b0 + bl
                        ps = ps_seg.tile([128, H], f32, name="ps_seg0", tag="ps_seg")
                        seg_mms(ps, mp_, bl, nb, "p", "0", cbase, 0, H2, True, True)
                        seg_mms(ps, mn_, bl, nb, "n", "0", cbase, H2, H2, True, True)
                        a0 = hwork.tile([128, H], f32, name="a0", tag="a0")
                        nc.vector.tensor_tensor(out=a0[:, :], in0=ps[:, :], in1=pre_tiles[b][:, :], op=OP.add)
                        r0 = hwork.tile([128, H], f32, name="r0", tag="r0")
                        nc.scalar.activation(out=r0[:, :], in_=a0[:, :], func=AF.Relu)
                        h0 = hwork.tile([128, H], f32, name="h0", tag="h0")
                        layer_norm(r0, sb["g0_bc"], sb["b0_bc"], h0)
                        ps_t = ps_tp.tile([128, 128], f32, name="ps_ht", tag="ps_t")
                        nc.tensor.transpose(out=ps_t[:, :], in_=h0[:, :], identity=sb["ident"][:, :])
                        hT = work.tile([128, 128], f32, name="hT", tag="hT")
                        nc.scalar.copy(out=hT[:, :], in_=ps_t[:, :])
                        ps_z = ps_dn.tile([128, H], f32, name="ps_z", tag="ps_dn")
                        nc.tensor.matmul(out=ps_z[:, :], lhsT=hT[:, :], rhs=sb["w_zp"][:, :], start=True, stop=True)
                        z_sb = work.tile([128, H], f32, name="z_sb", tag="z_sb")
                        nc.scalar.copy(out=z_sb[:, :], in_=ps_z[:, :])
                        nc.sync.dma_start(out=zp_own[b * 128 : (b + 1) * 128, :], in_=z_sb[:, :])
                        ps_z2 = ps_dn.tile([128, H], f32, name="ps_z2", tag="ps_dn")
                        nc.tensor.matmul(out=ps_z2[:, :], lhsT=hT[:, :], rhs=sb["w_zn"][:, :], start=True, stop=True)
                        z2_sb = work.tile([128, H], f32, name="z2_sb", tag="z_sb")
                        nc.scalar.copy(out=z2_sb[:, :], in_=ps_z2[:, :])
                        nc.sync.dma_start(out=zn_own[b * 128 : (b + 1) * 128, :], in_=z2_sb[:, :])
                        ps_pr1 = ps_dn.tile([128, H], f32, name="ps_pr1", tag="ps_dn")
                        nc.tensor.matmul(out=ps_pr1[:, :], lhsT=hT[:, :], rhs=sb["w_l1pr"][:, :], start=True, stop=True)
                        pre1_b = pre_pool.tile([128, H], f32, name="pre1_b", tag="pre")
                        nc.vector.tensor_tensor(out=pre1_b[:, :], in0=ps_pr1[:, :], in1=sb["bias1_bc"][:, :], op=OP.add)
                        pre_tiles[b] = pre1_b
                    cbase += nb * TPB

            # ---------------- phase E: allgather z
            nc.gpsimd.collective_compute(
                "AllGather", OP.bypass, replica_groups=rg,
                ins=[zp_own[:, :]], outs=[zp_full[:, :]],
            )
            nc.gpsimd.collective_compute(
                "AllGather", OP.bypass, replica_groups=rg,
                ins=[zn_own[:, :]], outs=[zn_full[:, :]],
            )

            # ---------------- phase F+G: layer-1 + proj + blend
            with tc.tile_pool(name="ctrl1", bufs=1) as ctrl1, tc.tile_pool(
                name="msgs1", bufs=1
            ) as msgs1:
                load_ctrl(ctrl1, "1")
                zsrc_p = [zp_full[bases[wi] : bases[wi] + sizes[wi], :] for wi in range(nw)]
                zsrc_n = [zn_full[bases[wi] : bases[wi] + sizes[wi], :] for wi in range(nw)]
                ofs_p = 0
                ofs_n = 0
                cbase = 0
                for (b0, nb) in chunks_all[1]:
                    mp_ = msgs1.tile([128, nb * TPB, H], f32, name="m1p", tag="m1p", bufs=2)
                    mn_ = msgs1.tile([128, nb * TPB, H], f32, name="m1n", tag="m1n", bufs=2)
                    ofs_p = gathers(mp_, zsrc_p, "idx_p1", ofs_p, nb, H)
                    ofs_n = gathers(mn_, zsrc_n, "idx_n1", ofs_n, nb, H)
                    for bl in range(nb):
                        b = b0 + bl
                        ps = ps_seg.tile([128, H], f32, name="ps_seg1", tag="ps_seg")
                        seg_mms(ps, mp_, bl, nb, "p", "1", cbase, 0, H, True, False)
                        seg_mms(ps, mn_, bl, nb, "n", "1", cbase, 0, H, False, True)
                        a1 = hwork.tile([128, H], f32, name="a1", tag="a0")
                        nc.vector.tensor_tensor(out=a1[:, :], in0=ps[:, :], in1=pre_tiles[b][:, :], op=OP.add)
                        r1 = hwork.tile([128, H], f32, name="r1", tag="r0")
                        nc.scalar.activation(out=r1[:, :], in_=a1[:, :], func=AF.Relu)
                        h1 = hwork.tile([128, H], f32, name="h1", tag="h0")
                        layer_norm(r1, sb["g1_bc"], sb["b1_bc"], h1)
                        ps_t = ps_tp.tile([128, 128], f32, name="ps_h1t", tag="ps_t")
                        nc.tensor.transpose(out=ps_t[:, :], in_=h1[:, :], identity=sb["ident"][:, :])
                        h1T = work.tile([128, 128], f32, name="h1T", tag="hT")
                        nc.scalar.copy(out=h1T[:, :], in_=ps_t[:, :])
                        ps_p1 = ps_dn.tile([128, H], f32, name="ps_p1", tag="ps_dn")
                        nc.tensor.matmul(out=ps_p1[:, :], lhsT=h1T[:, :], rhs=sb["w_p1"][:, :], start=True, stop=True)
                        tp_ = hwork.tile([128, H], f32, name="tp", tag="tp")
                        nc.vector.tensor_tensor(out=tp_[:, :], in0=ps_p1[:, :], in1=sb["pb1_bc"][:, :], op=OP.add)
                        gl = hwork.tile([128, H], f32, name="gl", tag="gl")
                        nc.scalar.activation(out=gl[:, :], in_=tp_[:, :], func=AF.Gelu)
                        ps_t2 = ps_tp.tile([128, 128], f32, name="ps_glt", tag="ps_t")
                        nc.tensor.transpose(out=ps_t2[:, :], in_=gl[:, :], identity=sb["ident"][:, :])
                        glT = work.tile([128, 128], f32, name="glT", tag="hT")
                        nc.scalar.copy(out=glT[:, :], in_=ps_t2[:, :])
                        ps_p2 = ps_dn.tile([128, H], f32, name="ps_p2", tag="ps_dn")
                        nc.tensor.matmul(out=ps_p2[:, :], lhsT=glT[:, :], rhs=sb["w_p2"][:, :], start=True, stop=True)
                        t2 = hwork.tile([128, H], f32, name="t2", tag="tp")
                        nc.vector.tensor_tensor(out=t2[:, :], in0=ps_p2[:, :], in1=sb["pb2_bc"][:, :], op=OP.add)
                        x2 = work.tile([128, H], f32, name="x2", tag="x_sb")
                        nc.sync.dma_start(out=x2, in_=x_in[b * 128 : (b + 1) * 128, :])
                        x2s = hwork.tile([128, H], f32, name="x2s", tag="gl")
                        nc.vector.tensor_scalar(
                            out=x2s[:, :], in0=x2[:, :],
                            scalar1=float(one_minus_gate), scalar2=None,
                            op0=OP.mult,
                        )
                        ob = hwork.tile([128, H], f32, name="ob", tag="ob")
                        nc.vector.tensor_tensor(out=ob[:, :], in0=t2[:, :], in1=x2s[:, :], op=OP.add)
                        nc.sync.dma_start(out=out_t[b * 128 : (b + 1) * 128, :], in_=ob[:, :])
                    cbase += nb * TPB

    nc.compile()
    return nc


def _run_spmd_traced(nc, in_maps, core_ids, tmpdir):
    """run_bass_kernel_spmd(trace=True) equivalent that does not require the
    antenv package: drives the NTFF hook from trn_agent_boot directly."""
    import glob as _glob

    from concourse import bass2jax, bass_utils

    try:
        from trn_agent_boot.trn_boot import _ntff_profile_via_ctypes

        hook = _ntff_profile_via_ctypes("/opt/axon/libaxon_pjrt.so")
    except Exception:
        hook = None
    if hook is None:
        results = bass2jax.run_bass_via_pjrt(nc, in_maps, n_cores=len(core_ids))
        return bass_utils.BassKernelResults(
            results=results, instructions_and_trace=None,
            profile_json=None, exec_time_ns=None,
        )
    import shutil

    shutil.rmtree(tmpdir, ignore_errors=True)
    os.makedirs(tmpdir, exist_ok=True)
    with hook(tmpdir, [0]):
        results = bass2jax.run_bass_via_pjrt(nc, in_maps, n_cores=len(core_ids))
    ntffs = _glob.glob(os.path.join(tmpdir, "*_body*.ntff"))
    if not ntffs:
        return bass_utils.BassKernelResults(
            results=results, instructions_and_trace=None,
            profile_json=None, exec_time_ns=None,
        )
    try:
        import gauge.profiler
        from concourse.bass_utils import FishPath, _process_ntff_profile

        profile = gauge.profiler.Profile(
            profile_path=FishPath(tmpdir),
            kernel_dev_mode=True,
            profile_on_exit=False,
            bass_kernel=nc.m,
            offline_processing=True,
            fname="*_body*",
            metadata={},
        )
        return _process_ntff_profile(
            profile, tmpdir, nc, core_ids, None, False, {}, trace_events=False
        ).as_bass_kernel_results(results)
    except Exception as e:
        print(f"[sgcn] profile processing failed: {type(e).__name__}: {e}")
        return bass_utils.BassKernelResults(
            results=results, instructions_and_trace=None,
            profile_json=None, exec_time_ns=None,
        )


# ---------------------------------------------------------------------------
# driver
# ---------------------------------------------------------------------------
def _run(inputs, cfg, use_sim=False):
    global LAST_EXEC_NS, LAST_RESULTS
    in_maps, perm, omg = _prep(inputs, cfg)
    nc = _build(cfg, omg)
    NC = cfg["NC"]

    if use_sim:
        from concourse.bass_interp import MultiCoreSim

        sim = MultiCoreSim(nc, NC)
        for c in range(NC):
            for k, v in in_maps[c].items():
                sim.cores[c].tensor(k)[:] = v
        sim.simulate()
        outs = [np.array(sim.cores[c].tensor("out")) for c in range(NC)]
    else:
        from concourse import bass_utils

        trace = bool(os.environ.get("SGCN_TRACE"))
        tmpdir = os.environ.get("SGCN_TRACE_DIR") or "/tmp/sgcn_trace"
        if trace:
            res = _run_spmd_traced(nc, in_maps, list(range(NC)), tmpdir)
        else:
            res = bass_utils.run_bass_kernel_spmd(nc, in_maps, list(range(NC)))
        LAST_EXEC_NS = res.exec_time_ns
        LAST_RESULTS = res
        outs = [np.asarray(res.results[c]["out"]) for c in range(NC)]

    full = np.concatenate(outs, axis=0)
    out = full[perm[: cfg["N"]]]
    return np.ascontiguousarray(out.astype(np.float32))


def kernel(**inputs) -> np.ndarray:
    return _run(inputs, FULL_CFG, use_sim=False)
